# revision 20
# baseline (speedup 1.0000x reference)
"""Trainium2 Bass kernel for sparse-conv + BN + ReLU, Z-table formulation.

Key idea: the per-(voxel,offset) contribution feats[nbr[i,k]] @ W[k] is a
row of the precomputed table Z[j*27+k] = feats[j] @ W[k] (bf16, built on
device: 7813 PE matmuls + 1.7 GB HBM writes). Gathering from Z makes
descriptors k-agnostic, so the ~50% masked pairs can be PACKED OUT on the
host: voxels are sorted by valid-neighbor count and dealt into 128-voxel
tiles whose slot budget C_t is the max count in the tile (~13.4k gather
instructions/core total vs 26,460 for the direct formulation — the Pool
engine's ~1.16us/indirect-DMA is the bottleneck). Each gather instruction
fetches 128 rows of Z into one 32-column slot block; a strided-view DVE
reduce sums the slots into the SBUF-resident y tile. Padding slots read a
zero row of Z.

y layout is voxel-on-partition [128, 980*32], so BN stats use a
strided-view DVE reduce + ones-vector PE matmul for the partition axis,
the per-channel scale/shift is broadcast via a PE outer product, applied
with DVE, and the store is a direct (transpose-free) DMA. The host
un-permutes the sorted voxel order afterwards.
"""
import contextlib
import ctypes
import os
import sys
import types

os.environ["NEURON_SCRATCHPAD_PAGE_SIZE"] = "2048"

import ml_dtypes
import numpy as np

import concourse.bass as bass
import concourse.bacc as bacc
import concourse.tile as tile
from concourse import mybir
from concourse.masks import make_identity

P = 128
K = 27
CIN = COUT = 32
NROWS = 1_000_000
EPS = 1e-5
NCORES = 8
T_TILES = 980              # 980*128 = 125440 >= 125000 per core
NJT = 7813                 # ceil((NROWS+63)/128): j-tiles for Z build
NJROWS = NJT * P           # 1000064 feats rows incl. zero pad
ZROWS = NJROWS * K         # 27001728 Z rows
ZROW = NROWS * K           # 27000000: zero row (j=1M is zero-padded)
FCH = 32                   # j-tiles per feats load chunk
ZB = 4                     # j-tiles per Z writeback
JT0 = 2734                 # j-tiles in stage 0 (~35%); Z rows shifted by +8
JSPLIT = JT0 * P * K       # stage-0 jk boundary
dt = mybir.dt
Alu = mybir.AluOpType
Act = mybir.ActivationFunctionType


def _build(sched0, sched1, n_total):
    n_tiles = len(sched0)
    totc = int(sum(sched0) + sum(sched1))
    nc = bacc.Bacc("TRN2", num_devices=NCORES)
    feats_b = nc.declare_dram_parameter(
        "feats_b", [NJROWS, CIN], dt.bfloat16, isOutput=False)
    wall_d = nc.declare_dram_parameter(
        "wall8", [P, 2 * K * COUT], dt.bfloat16, isOutput=False)
    pk_d = nc.declare_dram_parameter(
        "pk", [P, totc], dt.int32, isOutput=False)
    gam_d = nc.declare_dram_parameter(
        "gamma_r", [1, COUT], dt.float32, isOutput=False)
    bet_d = nc.declare_dram_parameter(
        "beta_r", [1, COUT], dt.float32, isOutput=False)
    ones_d = nc.declare_dram_parameter(
        "ones_r", [1, P], dt.float32, isOutput=False)
    out_d = nc.declare_dram_parameter(
        "out_sh", [P, n_tiles * COUT], dt.float32, isOutput=True)

    ycols = n_tiles * COUT

    with tile.TileContext(nc) as tc:
        with tc.tile_pool(name="const", bufs=1) as cpool, \
             tc.tile_pool(name="ysb", bufs=1) as ypool, \
             tc.tile_pool(name="ft", bufs=3) as fpool, \
             tc.tile_pool(name="tr", bufs=3) as trpool, \
             tc.tile_pool(name="zw", bufs=3) as zwpool, \
             tc.tile_pool(name="pkp", bufs=4) as pkpool, \
             tc.tile_pool(name="ga", bufs=6) as apool, \
             tc.tile_pool(name="tmp", bufs=4) as tmpool, \
             tc.tile_pool(name="trp", bufs=2, space="PSUM") as trppool, \
             tc.tile_pool(name="zp", bufs=4, space="PSUM") as zppool, \
             tc.tile_pool(name="sp", bufs=2, space="PSUM") as sppool, \
             tc.tile_pool(name="st", bufs=2) as stpool, \
             tc.tile_pool(name="zd", bufs=1, space="DRAM") as zdpool, \
             tc.tile_pool(name="dram", bufs=1, space="DRAM") as dpool:

            identb = cpool.tile([P, P], dt.bfloat16)
            make_identity(nc, identb[:])
            wall8 = cpool.tile([P, 2 * K * COUT], dt.bfloat16)
            nc.sync.dma_start(out=wall8[:], in_=wall_d[:])
            gam = cpool.tile([1, COUT], dt.float32)
            nc.sync.dma_start(out=gam[:], in_=gam_d[:])
            bet = cpool.tile([1, COUT], dt.float32)
            nc.sync.dma_start(out=bet[:], in_=bet_d[:])
            ones_r = cpool.tile([1, P], dt.float32)
            nc.sync.dma_start(out=ones_r[:], in_=ones_d[:])
            ones_c = cpool.tile([P, 1], dt.float32)
            nc.vector.memset(ones_c[:], 1.0)

            zt = zdpool.tile([ZROWS + 8, CIN], dt.bfloat16)
            zrow = cpool.tile([8, CIN], dt.bfloat16)
            nc.vector.memset(zrow[:], 0)
            nc.sync.dma_start(out=zt[0:8, :], in_=zrow[:])

            # ---- phase A (staged): Z = feats @ W_all ----
            def phase_a(jt0, jt1, act_only):
                for ch0 in range(jt0, jt1, FCH):
                    ch1 = min(ch0 + FCH, jt1)
                    nt = ch1 - ch0
                    ft = fpool.tile([P, FCH * CIN], dt.bfloat16, tag="ft")
                    nc.sync.dma_start(
                        out=ft[:, :nt * CIN].rearrange(
                            "p (t c) -> p t c", c=CIN),
                        in_=feats_b[ch0 * P:ch1 * P, :].rearrange(
                            "(t p) c -> p t c", p=P))
                    for b0 in range(0, nt, 4):
                        b1 = min(b0 + 4, nt)
                        nb = b1 - b0
                        kw = nb * CIN
                        trp = trppool.tile([P, P], dt.bfloat16, tag="trp")
                        nc.tensor.transpose(
                            out=trp[:kw, :],
                            in_=ft[:, b0 * CIN:b0 * CIN + kw],
                            identity=identb[:])
                        trs = trpool.tile([P, P], dt.bfloat16, tag="tr")
                        nc.scalar.copy(out=trs[:kw, :], in_=trp[:kw, :])
                        zw = zwpool.tile([P, ZB * K * COUT], dt.bfloat16,
                                         tag="zw")
                        hw = K * COUT // 2
                        for pp in range(0, (nb + 1) // 2):
                            po = 2 * CIN * pp
                            if 2 * pp + 1 < nb:
                                nq, cw2 = 4, 2 * CIN
                            else:
                                nq, cw2 = 2, CIN
                            for q in range(nq):
                                zp = zppool.tile([P, hw], dt.float32,
                                                 tag="zp")
                                nc.tensor.matmul(
                                    out=zp[:],
                                    lhsT=trs[po:po + cw2, :],
                                    rhs=wall8[po:po + cw2,
                                              q * hw:(q + 1) * hw],
                                    start=True, stop=True,
                                    tile_position=(po, 0),
                                )
                                zwo = zw[:, 2 * pp * K * COUT + q * hw:
                                         2 * pp * K * COUT + (q + 1) * hw]
                                if act_only or q % 2 == 0:
                                    nc.scalar.copy(out=zwo, in_=zp[:])
                                else:
                                    nc.vector.tensor_copy(out=zwo, in_=zp[:])
                        nc.sync.dma_start(
                            out=zt[8 + (ch0 + b0) * P * K:
                                   8 + (ch0 + b1) * P * K, :]
                            .rearrange("(b p k) c -> p b k c", b=nb, p=P),
                            in_=zw[:, :nb * K * COUT].rearrange(
                                "p (b k c) -> p b k c", b=nb, c=CIN))

            # ---- phase B (staged): packed gather-sum from Z ----
            y_sb = ypool.tile([P, ycols], dt.float32)

            def phase_b(schedule, cb0, zrows, accumulate):
                chunks = []
                cur = []
                cw = 0
                for t in range(n_tiles):
                    if cw + schedule[t] > 544 and cur:
                        chunks.append(cur)
                        cur, cw = [], 0
                    cur.append(t)
                    cw += schedule[t]
                if cur:
                    chunks.append(cur)
                cb = cb0
                for tl_list in chunks:
                    ccols = int(sum(schedule[t] for t in tl_list))
                    if ccols == 0:
                        continue
                    pk = pkpool.tile([P, 576], dt.int32, tag="pk")
                    nc.sync.dma_start(
                        out=pk[:, :ccols], in_=pk_d[:, cb:cb + ccols])
                    lc = 0
                    for t in tl_list:
                        ct = int(schedule[t])
                        if ct == 0:
                            continue
                        ga = apool.tile([P, K * COUT], dt.bfloat16, tag="ga")
                        for c in range(ct):
                            nc.gpsimd.indirect_dma_start(
                                out=ga[:, c * COUT:(c + 1) * COUT],
                                out_offset=None,
                                in_=zt[0:zrows, :],
                                in_offset=bass.IndirectOffsetOnAxis(
                                    ap=pk[:, lc + c:lc + c + 1], axis=0),
                            )
                        gav = ga[:, :ct * COUT].rearrange(
                            "p (n c) -> p c n", c=COUT)
                        if not accumulate:
                            nc.vector.tensor_reduce(
                                out=y_sb[:, t * COUT:(t + 1) * COUT],
                                in_=gav, axis=mybir.AxisListType.X,
                                op=Alu.add)
                        else:
                            tmp = tmpool.tile([P, COUT], dt.float32,
                                              tag="tmp")
                            nc.vector.tensor_reduce(
                                out=tmp[:], in_=gav,
                                axis=mybir.AxisListType.X, op=Alu.add)
                            nc.vector.tensor_tensor(
                                out=y_sb[:, t * COUT:(t + 1) * COUT],
                                in0=y_sb[:, t * COUT:(t + 1) * COUT],
                                in1=tmp[:], op=Alu.add)
                        lc += ct
                    cb += ccols

            phase_a(0, JT0, act_only=False)
            nc.vector.memset(y_sb[:], 0)
            phase_b(sched0, 0, 8 + JSPLIT, accumulate=False)
            phase_a(JT0, NJT, act_only=True)
            phase_b(sched1, int(sum(sched0)), ZROWS + 8, accumulate=True)

            # ---- BN stats ----
            r1 = cpool.tile([P, COUT], dt.float32)
            nc.vector.tensor_reduce(
                out=r1[:], in_=y_sb[:].rearrange("p (n c) -> p c n", c=COUT),
                axis=mybir.AxisListType.X, op=Alu.add)
            r2 = cpool.tile([P, COUT], dt.float32)
            scr = cpool.tile([P, 512], dt.float32)
            r2c = cpool.tile([P, COUT], dt.float32)
            for ci, c0 in enumerate(range(0, ycols, 512)):
                c1 = min(c0 + 512, ycols)
                nc.scalar.activation(
                    out=scr[:, :c1 - c0], in_=y_sb[:, c0:c1],
                    func=Act.Square)
                nc.vector.tensor_reduce(
                    out=(r2 if ci == 0 else r2c)[:],
                    in_=scr[:, :c1 - c0].rearrange(
                        "p (n c) -> p c n", c=COUT),
                    axis=mybir.AxisListType.X, op=Alu.add)
                if ci > 0:
                    nc.vector.tensor_tensor(
                        out=r2[:], in0=r2[:], in1=r2c[:], op=Alu.add)
            r12 = cpool.tile([P, 2 * COUT], dt.float32)
            nc.vector.tensor_copy(out=r12[:, :COUT], in_=r1[:])
            nc.vector.tensor_copy(out=r12[:, COUT:], in_=r2[:])
            sp = sppool.tile([1, 2 * COUT], dt.float32, tag="sp")
            nc.tensor.matmul(out=sp[:], lhsT=ones_c[:], rhs=r12[:],
                             start=True, stop=True)
            s12 = cpool.tile([1, 2 * COUT], dt.float32)
            nc.scalar.copy(out=s12[:], in_=sp[:])

            cc_in = dpool.tile([1, 2 * COUT], dt.float32)
            cc_out = dpool.tile([1, 2 * COUT], dt.float32)
            nc.sync.dma_start(out=cc_in[:], in_=s12[:])
            nc.gpsimd.collective_compute(
                "AllReduce", Alu.add,
                replica_groups=[list(range(NCORES))],
                ins=[cc_in.opt()], outs=[cc_out.opt()])
            s12r = cpool.tile([1, 2 * COUT], dt.float32)
            nc.sync.dma_start(out=s12r[:], in_=cc_out[:])

            mv = cpool.tile([1, 2 * COUT], dt.float32)
            nc.vector.tensor_scalar_mul(mv[:], s12r[:], 1.0 / n_total)
            mean2 = cpool.tile([1, COUT], dt.float32)
            nc.vector.tensor_tensor(
                out=mean2[:], in0=mv[:, :COUT], in1=mv[:, :COUT],
                op=Alu.mult)
            var = cpool.tile([1, COUT], dt.float32)
            nc.vector.tensor_tensor(
                out=var[:], in0=mv[:, COUT:], in1=mean2[:], op=Alu.subtract)
            nc.vector.tensor_scalar_add(var[:], var[:], EPS)
            std = cpool.tile([1, COUT], dt.float32)
            nc.scalar.activation(out=std[:], in_=var[:], func=Act.Sqrt)
            rstd = cpool.tile([1, COUT], dt.float32)
            nc.vector.reciprocal(out=rstd[:], in_=std[:])
            sc_row = cpool.tile([1, COUT], dt.float32)
            nc.vector.tensor_tensor(
                out=sc_row[:], in0=gam[:], in1=rstd[:], op=Alu.mult)
            sh_row = cpool.tile([1, COUT], dt.float32)
            nc.vector.tensor_tensor(
                out=sh_row[:], in0=mv[:, :COUT], in1=sc_row[:], op=Alu.mult)
            nc.vector.tensor_tensor(
                out=sh_row[:], in0=bet[:], in1=sh_row[:], op=Alu.subtract)

            # broadcast rows -> [P, 32] via outer product, then tile to 512
            ssp = sppool.tile([P, 2 * COUT], dt.float32, tag="sp")
            nc.tensor.matmul(out=ssp[:, :COUT], lhsT=ones_r[:],
                             rhs=sc_row[:], start=True, stop=True)
            nc.tensor.matmul(out=ssp[:, COUT:], lhsT=ones_r[:],
                             rhs=sh_row[:], start=True, stop=True)
            sc_rep = cpool.tile([P, 512], dt.float32)
            sh_rep = cpool.tile([P, 512], dt.float32)
            for r in range(512 // COUT):
                nc.scalar.copy(
                    out=sc_rep[:, r * COUT:(r + 1) * COUT], in_=ssp[:, :COUT])
                nc.scalar.copy(
                    out=sh_rep[:, r * COUT:(r + 1) * COUT], in_=ssp[:, COUT:])

            # ---- pass 2: y = relu(y*scale + shift), store ----
            for c0 in range(0, ycols, 512):
                c1 = min(c0 + 512, ycols)
                w = c1 - c0
                nc.vector.tensor_tensor(
                    out=y_sb[:, c0:c1], in0=y_sb[:, c0:c1],
                    in1=sc_rep[:, :w], op=Alu.mult)
                nc.vector.tensor_tensor(
                    out=y_sb[:, c0:c1], in0=y_sb[:, c0:c1],
                    in1=sh_rep[:, :w], op=Alu.add)
                nc.vector.tensor_scalar_max(y_sb[:, c0:c1], y_sb[:, c0:c1],
                                            0.0)
            for c0 in range(0, ycols, 4096):
                c1 = min(c0 + 4096, ycols)
                nc.sync.dma_start(out=out_d[:, c0:c1], in_=y_sb[:, c0:c1])
    return nc


def _install_ntff_hook():
    if "antenv.axon_hooks" in sys.modules:
        return
    try:
        lib = ctypes.CDLL("/opt/axon/libaxon_pjrt.so")
        lib.axon_start_nrt_profile.argtypes = [
            ctypes.POINTER(ctypes.c_int64), ctypes.c_size_t]
        lib.axon_start_nrt_profile.restype = ctypes.c_int64
        lib.axon_stop_nrt_profile.argtypes = [ctypes.c_char_p]
        lib.axon_stop_nrt_profile.restype = ctypes.c_int64
    except OSError:
        return

    @contextlib.contextmanager
    def _hook(output_dir, device_ids):
        import jax
        jax.devices()
        if device_ids:
            ids = (ctypes.c_int64 * len(device_ids))(*device_ids)
            rc = lib.axon_start_nrt_profile(ids, len(device_ids))
        else:
            rc = lib.axon_start_nrt_profile(None, 0)
        if rc != 0:
            raise RuntimeError(f"axon_start_nrt_profile rc={rc}")
        try:
            yield
        finally:
            n = lib.axon_stop_nrt_profile(str(output_dir).encode())
            if n <= 0:
                print(f"profile: {n} files in {output_dir}", file=sys.stderr)

    mod = types.ModuleType("antenv.axon_hooks")
    mod.get_axon_ntff_profile_hook = lambda: _hook
    mod.set_axon_ntff_profile_hook = lambda h: None
    sys.modules["antenv.axon_hooks"] = mod


_NC_CACHE = {}


def _get_nc(sched0, sched1):
    key = (tuple(sched0), tuple(sched1))
    if key not in _NC_CACHE:
        nc = _build(sched0, sched1, NROWS)
        nc.finalize()
        _NC_CACHE[key] = nc
    return _NC_CACHE[key]


def kernel(feats, W, gamma, beta, nbr_idx, nbr_mask, trace=False):
    feats = np.asarray(feats, np.float32)
    W = np.asarray(W, np.float32)
    gamma = np.asarray(gamma, np.float32)
    beta = np.asarray(beta, np.float32)
    nbr_idx = np.asarray(nbr_idx, np.int32)
    nbr_mask = np.asarray(nbr_mask, bool)
    n = feats.shape[0]
    assert n == NROWS and n % NCORES == 0
    per_core = n // NCORES

    feats_b = np.zeros((NJROWS, CIN), ml_dtypes.bfloat16)
    feats_b[:n] = feats.astype(ml_dtypes.bfloat16)
    # block-diagonal weight bands for tile-pair (64-contraction) matmuls:
    # rows [64p .. 64p+32) carry W at cols 0..863, rows [64p+32 .. 64p+64)
    # carry W at cols 864..1727
    wall = np.ascontiguousarray(
        W.transpose(1, 0, 2).reshape(CIN, K * COUT))
    wall8 = np.zeros((P, 2 * K * COUT), np.float32)
    for p_ in range(2):
        wall8[64 * p_:64 * p_ + 32, :K * COUT] = wall
        wall8[64 * p_ + 32:64 * p_ + 64, K * COUT:] = wall
    wall8 = wall8.astype(ml_dtypes.bfloat16)
    gamma_r = gamma.reshape(1, COUT)
    beta_r = beta.reshape(1, COUT)
    ones_r = np.ones((1, P), np.float32)

    # packed jk indices sorted ascending per voxel; +8 row shift so row 0
    # is a guaranteed zero row usable as padding in both stages
    jk = nbr_idx * np.int32(K) + np.arange(K, dtype=np.int32)[None, :]
    jkp = np.where(nbr_mask, jk, np.int32(ZROW))
    jkp = np.sort(jkp, axis=1)
    counts = nbr_mask.sum(1).astype(np.int32)
    h0 = (jkp < JSPLIT).sum(1).astype(np.int32)
    jkp8 = (jkp + 8).astype(np.int32)

    orders = []
    cts0 = np.zeros((NCORES, T_TILES), np.int32)
    cts1 = np.zeros((NCORES, T_TILES), np.int32)
    npad = T_TILES * P - per_core
    for c in range(NCORES):
        lo = c * per_core
        cs = counts[lo:lo + per_core]
        hs = h0[lo:lo + per_core]
        order = np.lexsort((-hs, -cs))
        orders.append(order)
        h0s = np.concatenate(
            [hs[order], np.zeros(npad, np.int32)]).reshape(T_TILES, P)
        h1s = np.concatenate(
            [(cs - hs)[order], np.zeros(npad, np.int32)]).reshape(T_TILES, P)
        cts0[c] = h0s.max(1)
        cts1[c] = h1s.max(1)
    sched0 = cts0.max(0)
    sched1 = cts1.max(0)
    tot0 = int(sched0.sum())
    totc = tot0 + int(sched1.sum())
    cb0 = np.concatenate([[0], np.cumsum(sched0)[:-1]]).astype(np.int64)
    cb1 = tot0 + np.concatenate(
        [[0], np.cumsum(sched1)[:-1]]).astype(np.int64)

    in_maps = []
    for c in range(NCORES):
        lo = c * per_core
        ord_ = orders[c]
        Rj = np.zeros((T_TILES * P, K), np.int32)
        Rj[:per_core] = jkp8[lo:lo + per_core][ord_]
        Rh = np.zeros(T_TILES * P, np.int32)
        Rh[:per_core] = h0[lo:lo + per_core][ord_]
        Rc = np.zeros(T_TILES * P, np.int32)
        Rc[:per_core] = counts[lo:lo + per_core][ord_]
        R3 = Rj.reshape(T_TILES, P, K)
        H3 = Rh.reshape(T_TILES, P)
        C3 = Rc.reshape(T_TILES, P)
        pk = np.zeros((P, totc), np.int32)
        for t in range(T_TILES):
            s0 = int(sched0[t])
            s1 = int(sched1[t])
            if s0:
                m0 = np.arange(s0)[None, :] < H3[t][:, None]
                pk[:, cb0[t]:cb0[t] + s0] = np.where(m0, R3[t][:, :s0], 0)
            if s1:
                ix = H3[t][:, None] + np.arange(s1)[None, :]
                ok = ix < C3[t][:, None]
                ix = np.minimum(ix, K - 1)
                pk[:, cb1[t]:cb1[t] + s1] = np.where(
                    ok, np.take_along_axis(R3[t], ix, 1), 0)
        in_maps.append(dict(
            feats_b=feats_b, wall8=wall8, pk=pk,
            gamma_r=gamma_r, beta_r=beta_r, ones_r=ones_r))

    _install_ntff_hook()
    from concourse import bass_utils
    bass_utils.upload_artifacts = lambda tmpdir: tmpdir
    nc = _get_nc(sched0.tolist(), sched1.tolist())
    res = bass_utils.run_bass_kernel_spmd(
        nc, in_maps, core_ids=list(range(NCORES)), trace=trace)

    chunks = []
    for c in range(NCORES):
        o = res.results[c]["out_sh"].reshape(P, T_TILES, COUT)
        ys = o.transpose(1, 0, 2).reshape(T_TILES * P, COUT)
        rc = np.empty((per_core, COUT), np.float32)
        rc[orders[c]] = ys[:per_core]
        chunks.append(rc)
    out = np.concatenate(chunks, axis=0)
    if trace:
        kernel.last_exec_time_ns = res.exec_time_ns
        kernel.last_trace = (res.instructions_and_trace or (None, None))[1]
    return out
